# revision 23
# baseline (speedup 1.0000x reference)
"""PillarEncoder Trainium2 kernel.

Strategy (spatially-tiled sharding per hint):
- 8 cores; core k owns batch k//4, gy rows [(k%4)*128, (k%4)*128+128).
- Host shards points into per-core/per-row slot arrays (padded to per-row
  tile counts, uniform across cores), computes per-point cell scale
  1/(count+1e-6).
- Device per core: MLP 8->65 (bias/BN folded, ones-channel trick) -> 65->256,
  relu+clip, then segment-sum per gy-row via one-hot matmuls:
    row_accum[ch_blk, 512 cells] += featT[128pts, 128ch].T @ onehot[128pts, 512]
  where onehot[p, c] = (gx_p == c) * scale_p  (mean folded in).
  PSUM accumulates over the row's tiles; drained to SBUF staging
  (split across ACT/DVE), DMA'd out 8 rows at a time.
"""

import os

import numpy as np

import concourse.bass as bass
import concourse.bacc as bacc_mod
import concourse.mybir as mybir
import concourse.tile as tile
from concourse.bass_utils import run_bass_kernel_spmd

X0 = Y0 = np.float32(-51.2)
VS = np.float32(0.2)
GH = GW = 512
HP = GH * GW
CIN, HID, COUT = 4, 64, 256
BN_EPS = np.float32(1e-3)
NCORES = 8
ROWS_PER_CORE = 128
RG = 8  # rows per output DMA group
P = 128

F32 = mybir.dt.float32
F16 = mybir.dt.float16

LAST_RESULTS = None  # test.py reads exec_time_ns from here


def _fold_bn(W, b, g, be, m, v):
    s = (g.astype(np.float64) / np.sqrt(v.astype(np.float64) + 1e-3)).astype(
        np.float32
    )
    Wf = (s[:, None] * W).astype(np.float32)
    bf = (s * (b - m) + be).astype(np.float32)
    return Wf, bf


def _build_program(half_tiles, act_rows=5, clip_on=True):
    """One SPMD program; all 8 cores run it on their own inputs.

    half_tiles: [128][2] number of 128-point tiles per (local gy-row,
    half-row window), identical across cores (host takes the max).
    Points of a tile lie in one 256-cell half-window; the one-hot is
    built full-width (512) but the scatter matmul streams only that
    half-window's 256 columns.
    """
    half_tiles = np.asarray(half_tiles)
    assert half_tiles.shape == (ROWS_PER_CORE, 2)
    row_tiles = half_tiles.sum(axis=1)
    TILES = int(row_tiles.sum())
    SLOTS = TILES * P
    tile_base = np.zeros(ROWS_PER_CORE + 1, dtype=np.int64)
    tile_base[1:] = np.cumsum(row_tiles)
    AUG_CHUNK_ROWS = 16

    nc = bacc_mod.Bacc()
    aug_d = nc.dram_tensor("aug", [8, SLOTS], F32, kind="ExternalInput")
    cdat_d = nc.dram_tensor("cdat", [P, 512 + 2 * TILES], F32, kind="ExternalInput")
    w1t_d = nc.dram_tensor("w1t", [8, 65], F32, kind="ExternalInput")
    b1c_d = nc.dram_tensor("b1c", [65, 1], F32, kind="ExternalInput")
    w2t_d = nc.dram_tensor("w2t", [65, 256], F32, kind="ExternalInput")
    out_d = nc.dram_tensor("out", [256, ROWS_PER_CORE, 512], F32, kind="ExternalOutput")

    Relu = mybir.ActivationFunctionType.Relu
    Copy = mybir.ActivationFunctionType.Copy

    with tile.TileContext(nc) as tc:
        with (
            tc.tile_pool(name="const", bufs=1) as cpool,
            tc.tile_pool(name="augp", bufs=2) as apool,
            tc.tile_pool(name="hp", bufs=2) as hpool,
            tc.tile_pool(name="featp", bufs=4) as fpool,
            tc.tile_pool(name="ohp", bufs=4) as ohpool,
            tc.tile_pool(name="stagep", bufs=2) as spool,
            tc.tile_pool(name="ps_h", bufs=2, space="PSUM") as ps_h,
            tc.tile_pool(name="ps_f", bufs=2, space="PSUM") as ps_f,
            tc.tile_pool(name="ps_row", bufs=2, space="PSUM") as ps_row,
        ):
            w1t_s = cpool.tile([8, 65], F32)
            nc.sync.dma_start(out=w1t_s[:], in_=w1t_d[:])
            b1c_s = cpool.tile([65, 1], F32)
            nc.sync.dma_start(out=b1c_s[:], in_=b1c_d[:])
            w2t_s = cpool.tile([65, 256], F32)
            nc.sync.dma_start(out=w2t_s[:], in_=w2t_d[:])
            cdat_s = cpool.tile([P, 512 + 2 * TILES], F32)
            nc.sync.dma_start(out=cdat_s[:], in_=cdat_d[:])
            idx_s = cdat_s[:, 512 : 512 + TILES]
            sp_s = cdat_s[:, 512 + TILES :]

            iota16 = cpool.tile([P, 512], F16)
            nc.vector.tensor_copy(out=iota16[:], in_=cdat_s[:, :512])

            # PE has a single sync-wait slot per instruction; absorb the
            # weight-DMA waits with dummy matmuls so later matmuls wait on
            # one sem only.
            warm = ps_h.tile([P, 1], F32, tag="hps")
            nc.tensor.matmul(
                out=warm[:65, :], lhsT=w1t_s[:], rhs=w1t_s[:, :1],
                start=True, stop=True,
            )
            nc.tensor.matmul(
                out=warm[:, :], lhsT=w2t_s[:, :P], rhs=w2t_s[:, :1],
                start=True, stop=True,
            )
            # absorb b1c's DMA wait on ACT so h-relu waits on PE only
            scr = cpool.tile([65, 1], F32)
            nc.scalar.activation(out=scr[:], in_=b1c_s[:], func=Copy)

            for rg in range(ROWS_PER_CORE // RG):
                # rows 0..act_rows-1 drain via ACT, the rest via DVE;
                # separate tiles avoid cross-engine WAW waits on staging
                stage_a = spool.tile([P, act_rows * 1024], F32, tag="stage_a")
                stage_v = spool.tile([P, (RG - act_rows) * 1024], F32, tag="stage_v")
                for rloc in range(RG):
                    row = rg * RG + rloc
                    rt = row_tiles[row]
                    if row % AUG_CHUNK_ROWS == 0:
                        s0 = int(tile_base[row]) * P
                        s1 = int(tile_base[min(row + AUG_CHUNK_ROWS, ROWS_PER_CORE)]) * P
                        aug_s = apool.tile([8, s1 - s0], F32, tag="aug")
                        nc.sync.dma_start(out=aug_s[:], in_=aug_d[:, s0:s1])
                        aug_base = s0
                    ro = int(tile_base[row]) * P - aug_base

                    # --- MLP in <=4-subtile mm1 chunks (1 PSUM bank) ---
                    feats = []
                    for m0 in range(0, rt, 4):
                        mn = min(4, rt - m0)
                        h_ps = ps_h.tile([65, mn * P], F32, tag="hps")
                        nc.tensor.matmul(
                            out=h_ps[:],
                            lhsT=w1t_s[:],
                            rhs=aug_s[:, ro + m0 * P : ro + (m0 + mn) * P],
                            start=True,
                            stop=True,
                        )
                        h_s = hpool.tile([65, mn * P], F32, tag="h")
                        nc.scalar.activation(
                            out=h_s[:], in_=h_ps[:], func=Relu, bias=b1c_s[:]
                        )
                        # layer 2 in <=2-subtile chunks; feat -> f16
                        for c0 in range(0, mn, 2):
                            ns = min(2, mn - c0)
                            f_ps = ps_f.tile([P, ns * 256], F32, tag="fps")
                            for j in range(ns):
                                nc.tensor.matmul(
                                    out=f_ps[:, j * 256 : (j + 1) * 256],
                                    lhsT=h_s[:, (c0 + j) * P : (c0 + j + 1) * P],
                                    rhs=w2t_s[:],
                                    start=True,
                                    stop=True,
                                )
                            feat_s = fpool.tile([P, ns * 256], F16, tag="feat")
                            if clip_on:
                                nc.vector.tensor_scalar(
                                    out=feat_s[:],
                                    in0=f_ps[:],
                                    scalar1=0.0,
                                    scalar2=100.0,
                                    op0=mybir.AluOpType.max,
                                    op1=mybir.AluOpType.min,
                                )
                            else:
                                nc.scalar.activation(
                                    out=feat_s[:], in_=f_ps[:], func=Relu
                                )
                            feats.append(feat_s)

                    # --- scatter into row accumulator [128ch, 2*512] ---
                    # acc layout (cb, cell): tile t covers half hf ->
                    # matmul streams oh cols [hf*256, hf*256+256) only.
                    t0h = int(half_tiles[row][0])
                    acc = ps_row.tile([P, 1024], F32, tag="acc")
                    for t in range(rt):
                        hf = 0 if t < t0h else 1
                        first = t == (0 if hf == 0 else t0h)
                        last = t == ((t0h - 1) if hf == 0 else (rt - 1))
                        g = int(tile_base[row]) + t
                        w0 = hf * 256
                        oh = ohpool.tile([P, 256], F16, tag="oh")
                        nc.vector.tensor_scalar(
                            out=oh[:],
                            in0=iota16[:, w0 : w0 + 256],
                            scalar1=idx_s[:, g : g + 1],
                            scalar2=sp_s[:, g : g + 1],
                            op0=mybir.AluOpType.is_equal,
                            op1=mybir.AluOpType.mult,
                        )
                        fsrc = feats[t // 2]
                        j = t % 2
                        for cb in range(2):
                            nc.tensor.matmul(
                                out=acc[:, cb * 512 + w0 : cb * 512 + w0 + 256],
                                lhsT=fsrc[:, j * 256 + cb * P : j * 256 + cb * P + P],
                                rhs=oh[:],
                                start=first,
                                stop=last,
                            )

                    # --- drain PSUM -> staging (one [128,2,512] op) ---
                    # stage layout (cb, r, x): cols = cb*(rows*512) + r*512 + x
                    half = rloc if rloc < act_rows else rloc - act_rows
                    stg = stage_a if rloc < act_rows else stage_v
                    dst = stg[:].rearrange("p (cb rr) -> p cb rr", cb=2)[
                        :, :, half * 512 : (half + 1) * 512
                    ]
                    src = acc[:].rearrange("p (cb x) -> p cb x", cb=2)
                    if rloc < act_rows:
                        nc.scalar.activation(out=dst, in_=src, func=Copy)
                    else:
                        nc.vector.tensor_copy(out=dst, in_=src)

                # --- write out: one DMA per stage tile ---
                for r0, stg, nrows in (
                    (rg * RG, stage_a, act_rows),
                    (rg * RG + act_rows, stage_v, RG - act_rows),
                ):
                    nc.sync.dma_start(
                        out=out_d[:, r0 : r0 + nrows, :].rearrange(
                            "(cb p) r x -> p cb (r x)", cb=2
                        ),
                        in_=stg[:],
                    )

    nc.finalize()
    return nc


def _host_shard(points, W1, b1, g1, be1, m1, v1, W2, b2, g2, be2, m2, v2):
    B, N, _ = points.shape
    pts = np.asarray(points, dtype=np.float32)
    x, y, z, it = pts[..., 0], pts[..., 1], pts[..., 2], pts[..., 3]
    gx = np.trunc((x - X0) / VS).astype(np.int64)
    gy = np.trunc((y - Y0) / VS).astype(np.int64)
    valid = (gx >= 0) & (gx < GW) & (gy >= 0) & (gy < GH)

    W1f, b1f = _fold_bn(W1, b1, g1, be1, m1, v1)
    W2f, b2f = _fold_bn(W2, b2, g2, be2, m2, v2)
    W1e = W1f[:, [0, 1, 2, 3, 4, 5, 7, 8]] if W1f.shape[1] == 10 else W1f
    # aug channel order: x, y, z, i, x-cx, y-cy, cx, cy (zero chans dropped)

    # per-cell counts per batch (float32 like reference)
    counts = np.zeros((B, HP), dtype=np.float32)
    for b in range(B):
        m = valid[b]
        cell = (gy[b][m] * GW + gx[b][m]).astype(np.int64)
        np.add.at(counts[b], cell, np.float32(1.0))
    scale = np.float32(1.0) / (counts + np.float32(1e-6))

    # per-core (row, half) counts -> uniform per-(row,half) tile counts
    core_sel = []
    hc_all = np.zeros((NCORES, 2 * ROWS_PER_CORE), dtype=np.int64)
    for k in range(NCORES):
        b, r0 = k // 4, (k % 4) * ROWS_PER_CORE
        m = valid[b] & (gy[b] >= r0) & (gy[b] < r0 + ROWS_PER_CORE)
        idxs = np.nonzero(m)[0]
        rl = (gy[b][idxs] - r0).astype(np.int64)
        hl = rl * 2 + (gx[b][idxs] >= 256)
        hc = np.bincount(hl, minlength=2 * ROWS_PER_CORE)
        hc_all[k] = hc
        core_sel.append((b, r0, idxs, hl, hc))
    half_tiles = np.maximum(1, -(-hc_all.max(axis=0) // P)).reshape(
        ROWS_PER_CORE, 2
    )
    TILES = int(half_tiles.sum())
    SLOTS = TILES * P
    htile_base = np.zeros(2 * ROWS_PER_CORE + 1, dtype=np.int64)
    htile_base[1:] = np.cumsum(half_tiles.reshape(-1))

    cx = gx.astype(np.float32) * VS + X0 + VS / np.float32(2.0)
    cy = gy.astype(np.float32) * VS + Y0 + VS / np.float32(2.0)

    in_maps = []
    consts = {
        "w1t": np.zeros((8, 65), dtype=np.float32),
        "b1c": np.zeros((65, 1), dtype=np.float32),
        "w2t": np.zeros((65, 256), dtype=np.float32),
    }
    consts["w1t"][:, :64] = W1e.T
    consts["b1c"][:64, 0] = b1f
    consts["b1c"][64, 0] = 1.0
    consts["w2t"][:64, :] = W2f.T
    consts["w2t"][64, :] = b2f

    for k in range(NCORES):
        b, r0, idxs, hl, hc = core_sel[k]
        order = np.argsort(hl, kind="stable")
        oi = idxs[order]
        ohl = hl[order]
        offs = np.zeros(2 * ROWS_PER_CORE, dtype=np.int64)
        offs[1:] = np.cumsum(hc)[:-1]
        within = np.arange(len(oi)) - offs[ohl]
        slot = htile_base[ohl] * P + within

        aug = np.zeros((8, SLOTS), dtype=np.float32)
        aug[0, slot] = x[b][oi]
        aug[1, slot] = y[b][oi]
        aug[2, slot] = z[b][oi]
        aug[3, slot] = it[b][oi]
        aug[4, slot] = x[b][oi] - cx[b][oi]
        aug[5, slot] = y[b][oi] - cy[b][oi]
        aug[6, slot] = cx[b][oi]
        aug[7, slot] = cy[b][oi]

        idxloc = np.full(SLOTS, -1.0, dtype=np.float32)
        idxloc[slot] = gx[b][oi].astype(np.float32)
        sp = np.zeros(SLOTS, dtype=np.float32)
        cell = gy[b][oi] * GW + gx[b][oi]
        sp[slot] = scale[b][cell]

        cdat = np.empty((P, 512 + 2 * TILES), dtype=np.float32)
        cdat[:, :512] = np.arange(512, dtype=np.float32)[None, :]
        cdat[:, 512 : 512 + TILES] = idxloc.reshape(TILES, P).T
        cdat[:, 512 + TILES :] = sp.reshape(TILES, P).T

        im = dict(consts)
        im["aug"] = aug
        im["cdat"] = cdat
        in_maps.append(im)
    # interval bound on |feat| pre-clip: if < 100 the clip is inert and
    # the device can use a plain ACT relu instead of a DVE clamp.
    hmax = np.zeros(64, dtype=np.float64)
    for b in range(B):
        m = valid[b]
        a8 = np.stack(
            [x[b][m], y[b][m], z[b][m], it[b][m],
             x[b][m] - cx[b][m], y[b][m] - cy[b][m], cx[b][m], cy[b][m]]
        ).astype(np.float64)
        hb = np.maximum(W1e.astype(np.float64) @ a8 + b1f[:, None].astype(np.float64), 0.0)
        hmax = np.maximum(hmax, hb.max(axis=1) if hb.size else hmax)
    bound = np.abs(b2f.astype(np.float64)) + np.abs(W2f.astype(np.float64)) @ hmax
    clip_on = bool(bound.max() >= 99.0)

    return in_maps, half_tiles, clip_on


def kernel(points, W1, b1, g1, be1, m1, v1, W2, b2, g2, be2, m2, v2):
    global LAST_RESULTS
    in_maps, half_tiles, clip_on = _host_shard(
        points, W1, b1, g1, be1, m1, v1, W2, b2, g2, be2, m2, v2
    )
    nc = _build_program(half_tiles, act_rows=4, clip_on=True)
    trace = os.environ.get("PILLAR_TRACE", "0") == "1"
    res = run_bass_kernel_spmd(
        nc,
        in_maps,
        core_ids=list(range(NCORES)),
        trace=trace,
        trace_cores=[0] if trace else None,
    )
    LAST_RESULTS = res
    B = points.shape[0]
    out = np.empty((B, COUT, GH, GW), dtype=np.float32)
    for k in range(NCORES):
        b, r0 = k // 4, (k % 4) * ROWS_PER_CORE
        out[b, :, r0 : r0 + ROWS_PER_CORE, :] = res.results[k]["out"]
    return out
